# revision 4
# baseline (speedup 1.0000x reference)
"""Trainium2 Bass kernel for batched tiny-projection attention.

Reference computation (per batch b):
    qp = relu(q @ W1.T + b1)            [Nq, 3]
    kp = relu(k @ W2.T + b2)            [Nf, 3]
    scores = (qp @ kp.T) / sqrt(3)      [Nq, Nf]
    attn = softmax(scores, axis=-1)
    out = attn @ v                      [Nq, C]

Shapes: B=4, Nq=2048, Nf=16384, D=3, C=768, fp32.

Sharding: 8 cores = (4 batches) x (2 halves of Nq). Each core handles
q[b, h*1024:(h+1)*1024], full k[b]/v[b], so softmax is local to a core
(no cross-core reduction needed).

Device algorithm (per core), oriented for the tensor engine:
  - scores are computed TRANSPOSED: sT[m, n] = kp[m]. qp[n], because the
    attn @ v matmul needs the contraction dim (m) on partitions.
  - Exact fp32-grade scores at 1 cycle/row: PE matmul cost depends only
    on the moving free dim (N), not on K<=128. Each fp32 operand is
    split hi/lo into fp16 and the 4 cross products land on partition
    blocks {0,32,64,96} (kp: [hi,hi,lo,lo] x qp: [hi,lo,hi,lo]); unused
    partitions are exact zeros, so ONE K=128 matmul sums all 4 products.
  - The tiny projections run as K=9 fp16 matmuls (W hi/lo split) whose
    lhsT scatters the 3 output rows to the 4 partition blocks directly.
  - exp(scale*s - shift) runs on the scalar engine straight out of PSUM,
    emitting bf16 tiles (bf16 range avoids underflow for rows whose max
    score is far below the global shift; scores >= 0 since qp,kp >= 0).
  - attn @ v accumulates in PSUM over a group of m-tiles, then is
    flushed (added) into an SBUF fp32 accumulator; v carries an extra
    ones column so the softmax denominator falls out of the same matmul.
  - Final: out = acc[:, :768] * (1 / acc[:, 768]) per row, DMA to DRAM.
"""

import sys

sys.path.insert(0, "/opt/trn_rl_repo")

import numpy as np

import concourse.bass as bass
import concourse.bacc as bacc
import concourse.tile as tile
from concourse import mybir
from concourse.bass_utils import run_bass_kernel_spmd

F32 = mybir.dt.float32
F16 = mybir.dt.float16
BF16 = mybir.dt.bfloat16

B, NQ_FULL, NF, D, C = 4, 2048, 16384, 3, 768
SCALE = 1.0 / np.sqrt(3.0)
NQ = NQ_FULL // 2          # per-core query rows
CA, CB = 512, C + 1 - 512  # c-chunk split of [v | ones] (769 = 512 + 257)


def build_nc(nq=NQ, nf=NF, g=16, num_devices=8):
    """Build the single-core SPMD program. g = m-tiles (of 128) per group."""
    assert nq % 512 == 0 and nf % 128 == 0
    m_tiles = nf // 128
    assert m_tiles % g == 0
    ngroups = m_tiles // g
    nchunks = nq // 128
    gm = g * 128            # field rows per group
    assert gm % 512 == 0
    caug = C + 1

    nc = bacc.Bacc("TRN2", target_bir_lowering=False, debug=False,
                   num_devices=num_devices)

    qT9 = nc.dram_tensor("qT9", [9, nq], F16, kind="ExternalInput")
    kT9 = nc.dram_tensor("kT9", [9, nf], F16, kind="ExternalInput")
    vaug = nc.dram_tensor("vaug", [nf, caug], BF16, kind="ExternalInput")
    wq = nc.dram_tensor("wq", [9, 128], F16, kind="ExternalInput")
    wk = nc.dram_tensor("wk", [9, 128], F16, kind="ExternalInput")
    bq = nc.dram_tensor("bq", [128, 1], F32, kind="ExternalInput")
    bk = nc.dram_tensor("bk", [128, 1], F32, kind="ExternalInput")
    shift = nc.dram_tensor("shift", [128, 1], F32, kind="ExternalInput")
    out = nc.dram_tensor("out", [nq, C], F32, kind="ExternalOutput")

    BASES = (0, 32, 64, 96)

    with tile.TileContext(nc) as tc, \
         tc.tile_pool(name="const", bufs=1) as const, \
         tc.tile_pool(name="kio", bufs=2) as kio, \
         tc.tile_pool(name="kp32p", bufs=2) as kp32p, \
         tc.tile_pool(name="khip", bufs=2) as khip, \
         tc.tile_pool(name="ksplitp", bufs=2) as ksplitp, \
         tc.tile_pool(name="vp", bufs=2 * g) as vp, \
         tc.tile_pool(name="expp", bufs=2 * g) as expp, \
         tc.tile_pool(name="outp", bufs=2) as outp, \
         tc.tile_pool(name="recp", bufs=2) as recp, \
         tc.tile_pool(name="sc_ps", bufs=3, space="PSUM") as sc_ps, \
         tc.tile_pool(name="oA_ps", bufs=2, space="PSUM") as oA_ps, \
         tc.tile_pool(name="oB_ps", bufs=2, space="PSUM") as oB_ps, \
         tc.tile_pool(name="pj_ps", bufs=1, space="PSUM") as pj_ps:

        # ---- constants / once-per-core prologue ----
        wq_sb = const.tile([9, 128], F16)
        nc.sync.dma_start(wq_sb[:], wq[:])
        wk_sb = const.tile([9, 128], F16)
        nc.sync.dma_start(wk_sb[:], wk[:])
        bq_sb = const.tile([128, 1], F32)
        nc.sync.dma_start(bq_sb[:], bq[:])
        bk_sb = const.tile([128, 1], F32)
        nc.sync.dma_start(bk_sb[:], bk[:])
        shift_sb = const.tile([128, 1], F32)
        nc.sync.dma_start(shift_sb[:], shift[:])
        qT9_sb = const.tile([9, nq], F16)
        nc.sync.dma_start(qT9_sb[:], qT9[:])

        acc = const.tile([128, nchunks, caug], F32)

        def proj_and_split(w_sb, b_sb, rhs_sb, n, pool32, poolhi, poolsp,
                           lo_ranges):
            """Project rhs [9, n] -> p32 [128, n] (row blocks at BASES,
            zeros elsewhere), then build fp16 split tile with hi copies
            at hi_bases and lo residuals at lo_bases."""
            p32 = pool32.tile([128, n], F32)
            for h in range(n // 512):
                pj = pj_ps.tile([128, 512], F32)
                nc.tensor.matmul(pj[:], w_sb[:], rhs_sb[:, h * 512:(h + 1) * 512],
                                 start=True, stop=True)
                nc.scalar.activation(p32[:, h * 512:(h + 1) * 512], pj[:],
                                     mybir.ActivationFunctionType.Relu,
                                     bias=b_sb[:], scale=1.0)
            # full-tile fp16 round covers hi blocks AND keeps the zero
            # rows exact zeros (the K=128 scores matmul reads all rows);
            # then overwrite lo block ranges with the fp16 residuals.
            hsc = poolhi.tile([128, n], F16)
            sp = poolsp.tile([128, n], F16)
            nc.vector.tensor_copy(sp[:], p32[:])
            for p0, p1 in lo_ranges:
                nc.vector.tensor_copy(hsc[p0:p1, :], p32[p0:p1, :])
                nc.vector.tensor_sub(sp[p0:p1, :], p32[p0:p1, :],
                                     hsc[p0:p1, :])
            return sp

        # q: blocks [hi, lo, hi, lo];  k: blocks [hi, hi, lo, lo]
        qsplit = proj_and_split(wq_sb, bq_sb, qT9_sb, nq,
                                const, const, const,
                                lo_ranges=((32, 64), (96, 128)))

        def emit_projk(gi):
            kt = kio.tile([9, gm], F16)
            nc.sync.dma_start(kt[:], kT9[:, gi * gm:(gi + 1) * gm])
            return proj_and_split(wk_sb, bk_sb, kt, gm,
                                  kp32p, khip, ksplitp,
                                  lo_ranges=((64, 128),))

        def emit_v(gi):
            vts = []
            for t in range(g):
                m0 = (gi * g + t) * 128
                vt = vp.tile([128, caug], BF16)
                nc.sync.dma_start(vt[:], vaug[m0:m0 + 128, :])
                vts.append(vt)
            return vts

        def emit_scores(gi, ks, ts):
            """scores + exp for m-tiles ts (local idx) of group gi."""
            es = []
            for t in ts:
                et = expp.tile([128, nq], BF16)
                for h in range(nq // 512):
                    sp = sc_ps.tile([128, 512], F32)
                    nc.tensor.matmul(sp[:], ks[:, t * 128:(t + 1) * 128],
                                     qsplit[:, h * 512:(h + 1) * 512],
                                     start=True, stop=True)
                    nc.scalar.activation(et[:, h * 512:(h + 1) * 512], sp[:],
                                         mybir.ActivationFunctionType.Exp,
                                         bias=shift_sb[:], scale=float(SCALE))
                es.append(et)
            return es

        def emit_attn_chunk(gi, ci, es, vts):
            pA = oA_ps.tile([128, CA], F32)
            pB = oB_ps.tile([128, CB], F32)
            for i in range(g):
                e = es[i][:, ci * 128:(ci + 1) * 128]
                nc.tensor.matmul(pA[:], e, vts[i][:, 0:CA],
                                 start=(i == 0), stop=(i == g - 1))
                nc.tensor.matmul(pB[:], e, vts[i][:, CA:caug],
                                 start=(i == 0), stop=(i == g - 1))
            if gi == 0:
                nc.vector.tensor_copy(acc[:, ci, 0:CA], pA[:])
                nc.vector.tensor_copy(acc[:, ci, CA:caug], pB[:])
            else:
                nc.vector.tensor_add(acc[:, ci, 0:CA], acc[:, ci, 0:CA], pA[:])
                nc.vector.tensor_add(acc[:, ci, CA:caug], acc[:, ci, CA:caug],
                                     pB[:])

        # ---- software-pipelined main loop ----
        ks_cur = emit_projk(0)
        v_cur = emit_v(0)
        e_cur = emit_scores(0, ks_cur, range(g))
        for gi in range(ngroups):
            last = gi + 1 >= ngroups
            if not last:
                ks_nxt = emit_projk(gi + 1)
                v_nxt = emit_v(gi + 1)
                e_nxt = []
            # distribute next group's score matmuls across this group's
            # attn chunks to keep PE dense and ACT fed early
            per = (g + nchunks - 1) // nchunks
            for ci in range(nchunks):
                emit_attn_chunk(gi, ci, e_cur, v_cur)
                if not last:
                    ts = range(ci * per, min((ci + 1) * per, g))
                    e_nxt.extend(emit_scores(gi + 1, ks_nxt, ts))
            if not last:
                ks_cur, v_cur, e_cur = ks_nxt, v_nxt, e_nxt

        # ---- finale: normalize and store ----
        for ci in range(nchunks):
            rec = recp.tile([128, 1], F32)
            nc.vector.reciprocal(rec[:], acc[:, ci, C:caug])
            ot = outp.tile([128, C], F32)
            nc.vector.tensor_scalar_mul(ot[:], acc[:, ci, 0:C], rec[:])
            nc.sync.dma_start(out[ci * 128:(ci + 1) * 128, :], ot[:])

    nc.finalize()
    return nc


def _split16(x):
    hi = x.astype(np.float16)
    lo = (x - hi.astype(np.float32)).astype(np.float16)
    return hi, lo


def _wlhs(W):
    """lhsT [9, 128] for the projection matmul: K rows = [Whi, Whi, Wlo]
    (pairing rhs rows [xhi, xlo, xhi]); output cols 32c+e = projected
    row e replicated on the 4 partition blocks, zeros elsewhere."""
    Whi, Wlo = _split16(W.astype(np.float32))
    m = np.zeros((9, 128), np.float16)
    for e in range(3):
        for d in range(3):
            for cblk in range(4):
                m[0 + d, 32 * cblk + e] = Whi[e, d]
                m[3 + d, 32 * cblk + e] = Whi[e, d]
                m[6 + d, 32 * cblk + e] = Wlo[e, d]
    return m


def _brep(b):
    """bias [128, 1]: b[e] at partitions 32c+e, zero elsewhere."""
    m = np.zeros((128, 1), np.float32)
    for e in range(3):
        for cblk in range(4):
            m[32 * cblk + e, 0] = b[e]
    return m


def _t9(x2d):
    """[N, 3] -> [9, N] fp16 rows [hi, lo, hi]."""
    xT = np.ascontiguousarray(x2d.T.astype(np.float32))
    hi, lo = _split16(xT)
    return np.concatenate([hi, lo, hi], axis=0)


def _host_prep(q, k, v, W1, b1, W2, b2):
    """Build per-core input maps (layout/dtype prep only)."""
    import ml_dtypes
    wq_l, wk_l = _wlhs(W1), _wlhs(W2)
    bq_r, bk_r = _brep(b1), _brep(b2)

    in_maps = []
    per_batch = {}
    for b in range(B):
        # cheap per-batch upper bound on max score -> exp(s - shift) <= 1
        qp = np.maximum(q[b].astype(np.float32) @ W1.T.astype(np.float32)
                        + b1.astype(np.float32), 0.0)
        kp = np.maximum(k[b].astype(np.float32) @ W2.T.astype(np.float32)
                        + b2.astype(np.float32), 0.0)
        bound = SCALE * float(qp.max(axis=0) @ kp.max(axis=0))
        va = np.ones((NF, C + 1), np.float32)
        va[:, :C] = v[b]
        per_batch[b] = {
            "kT9": _t9(k[b]),
            "vaug": va.astype(ml_dtypes.bfloat16),
            "shift": np.full((128, 1), -bound, np.float32),
        }
    for core in range(8):
        b, h = core // 2, core % 2
        qs = q[b, h * NQ:(h + 1) * NQ, :]
        in_maps.append({
            "qT9": _t9(qs),
            "wq": wq_l, "wk": wk_l, "bq": bq_r, "bk": bk_r,
            **per_batch[b],
        })
    return in_maps


_NC_CACHE = {}


def kernel(q, k, v, W1, b1, W2, b2, _trace=False):
    q, k, v = np.asarray(q), np.asarray(k), np.asarray(v)
    W1, b1 = np.asarray(W1), np.asarray(b1)
    W2, b2 = np.asarray(W2), np.asarray(b2)

    if "nc" not in _NC_CACHE:
        _NC_CACHE["nc"] = build_nc()
    nc = _NC_CACHE["nc"]

    in_maps = _host_prep(q, k, v, W1, b1, W2, b2)
    res = run_bass_kernel_spmd(nc, in_maps, list(range(8)), trace=_trace)

    out = np.empty((B, NQ_FULL, C), np.float32)
    for core in range(8):
        b, h = core // 2, core % 2
        out[b, h * NQ:(h + 1) * NQ, :] = res.results[core]["out"]
    if _trace:
        return out, res
    return out
